# revision 23
# baseline (speedup 1.0000x reference)
"""Trainium2 Bass kernel for the DDSP decoder (nn_DDSPDecoder).

Sharding: pure data-parallel over batch B=8 across 8 NeuronCores; control-net
weights and reverb IR replicated; each core synthesizes one example.

Per-core pipeline (frames padded 1000->1024; GRU solved by quasi-linear Picard
iterations whose linear part is the hardware scan; harmonic phase factored
through the fundamental with a compensated two-float frame scan + Cody-Waite
range reduction; filtered noise done in the frequency domain with constant
DFT matrices; reverb as a 131072-pt 4-step FFT convolution on the PE).
"""
import os
import sys

if "/opt/trn_rl_repo" not in sys.path:
    sys.path.insert(0, "/opt/trn_rl_repo")

import numpy as np

import concourse.bass as bass  # noqa: F401
import concourse.tile as tile
from concourse import bacc, bass_utils, mybir
from concourse.alu_op_type import AluOpType as OP
from concourse import dve_ops as _dve_ops
from concourse.dve_spec import (Spec as _Spec, Src0 as _S0, Src1 as _S1,
                                C0 as _C0, C1 as _C1, Zero as _Zero,
                                One as _One, PageIdx as _PageIdx,
                                select as _select, lower as _dve_lower,
                                _has_src1 as _dve_has_src1)
from concourse.dve_uop import DveOpSpec as _DveOpSpec


def _register_dve(name, spec, subdim):
    for o in _dve_ops.OPS:
        if o.name == name:
            return o
    opcode = _dve_ops._CUSTOM_DVE_ROW_BASE + len(_dve_ops.OPS)
    shas = {}
    for ver in ("v3", "v4"):
        s = _DveOpSpec(name=name, opcode=opcode,
                       uops=_dve_lower(spec, ver=ver),
                       rd1_en=_dve_has_src1(spec))
        shas[ver] = s.sha(ver)
    op = _dve_ops.DveOp(name, spec, subdim=subdim, uops_sha=shas)
    _dve_ops.OPS.append(op)
    _dve_ops._SUB_OPCODE_FOR_NAME[name] = opcode
    _dve_ops.CUSTOM_DVE_SPECS[name] = spec
    return op


# f = u - round(u), u = (Src0 + C0) * Src1 ; s1 = MAGIC rounding constant.
_u = (_S0 + _C0) * _S1
OP_PHASE = _register_dve("DDSP_PHASE_FRAC",
                         _Spec(body=_u - ((_u + _C1) - _C1), reference=None),
                         subdim=False)
# ms = s16 if dpp*(h+1) < 1 else 0; pages = h, PageIdx(1,1) = h+1.
OP_MASK = _register_dve(
    "DDSP_MASK_SEL",
    _Spec(body=_select(_S0 * _PageIdx(_C0, _C1) < _One, _S1, _Zero),
          reference=None), subdim=True)
# ae = aC + (aN - aC) * (j/64); pages = j, PageIdx(0, 1/64) = j/64.
OP_AMP = _register_dve(
    "DDSP_AMP_INTERP",
    _Spec(body=_S0 + (_S1 - _S0) * _PageIdx(_C0, _C1), reference=None),
    subdim=True)


def _register_dve_perf(name, spec, subdim):
    op = _register_dve(name, spec, subdim)
    if not op.perf_en:
        op.perf_en.update({"v3": True, "v4": True})
    return op


OP_AMP2 = _register_dve_perf(
    "DDSP_AMP_INTERP2",
    _Spec(body=_S0 + (_S1 - _S0) * _PageIdx(_C0, _C1), reference=None),
    subdim=True)

F32 = mybir.dt.float32
F32R = mybir.dt.float32r
F16 = mybir.dt.bfloat16
FP16 = mybir.dt.float16
I32 = mybir.dt.int32
AFT = mybir.ActivationFunctionType

SR = 16000
T = 1000
TP = 1024
NCH = 8                      # frame chunks of 128
NS = 64000
NSP = 65536
H = 64
NB = 65
N_CORES = 8
GRU_ITERS = 4
LN10 = float(np.float32(np.log(np.float32(10.0))))
LN2 = float(np.float32(np.log(2.0)))
TWO_PI = 2.0 * np.pi
INV2PI = float(np.float32(1.0 / TWO_PI))
INV_PI = float(np.float32(1.0 / np.pi))
AX = mybir.AxisListType.X
MAGIC = 12582912.0  # 1.5 * 2**23: float32 round-to-nearest-int trick

DEBUG_TAPS = bool(int(os.environ.get("KERNEL_DEBUG_TAPS", "0")))


# ---------------------------------------------------------------------------
# host-side constants
# ---------------------------------------------------------------------------

def _cw_chunks(kmax_bits):
    nb = 24 - kmax_bits
    x = TWO_PI
    cs = []
    for _ in range(2):
        m, e = np.frexp(x)
        q = float(np.float32(np.ldexp(np.round(np.ldexp(m, nb)), e - nb)))
        cs.append(q)
        x -= q
    cs.append(float(np.float32(x)))
    return cs

CW_FRAME = _cw_chunks(12)
CW_SAMP = _cw_chunks(9)


def _noise_matrices():
    n = 128
    t = np.arange(n)
    C = np.zeros((NB, n))
    C[0] = 1.0 / n
    for m in range(1, 64):
        C[m] = 2.0 / n * np.cos(2 * np.pi * m * t / n)
    C[64] = 1.0 / n * np.cos(np.pi * t)
    w = 0.5 - 0.5 * np.cos(2.0 * np.pi * t / n)
    Cw = C * np.fft.fftshift(w)[None, :]
    Cc = np.zeros_like(Cw)
    Cc[:, (t + n // 2) % n] = Cw
    F = np.exp(-2j * np.pi * np.outer(t, np.arange(129)) / 256.0)
    Gc = Cc @ F
    Ff = np.exp(-2j * np.pi * np.outer(np.arange(64), np.arange(129)) / 256.0)
    tt = np.arange(256)
    DIre = np.zeros((129, 256))
    DIim = np.zeros((129, 256))
    for f in range(129):
        mult = 1.0 if f in (0, 128) else 2.0
        DIre[f] = mult * np.cos(2 * np.pi * f * tt / 256.0) / 256.0
        DIim[f] = -mult * np.sin(2 * np.pi * f * tt / 256.0) / 256.0
    return Gc, Ff, DIre, DIim


def _make_consts():
    f32 = np.float32
    c = {}
    j = np.arange(64, dtype=np.float64)
    sc = TWO_PI / SR
    c["w0ts"] = np.tile(f32((1.0 - j / 64.0) * sc), (128, 1))
    c["w1ts"] = np.tile(f32((j / 64.0) * sc), (128, 1))
    c["w0j"] = np.tile(f32(1.0 - j / 64.0), (128, 1))
    c["w1j"] = np.tile(f32(j / 64.0), (128, 1))
    h = np.arange(1, H + 1, dtype=np.float64)
    c["hrow"] = np.tile(f32(h), (128, 1))
    c["h2row"] = np.tile(f32(h / (2.0 * np.pi)), (128, 1))
    c["pioh"] = np.tile(f32(np.pi / h), (128, 1))
    c["c8000oh"] = np.tile(f32(8000.0 / h), (128, 1))
    c["eye"] = np.eye(128, dtype=np.float32)
    c["ones"] = np.ones((128, TP), np.float32)
    Gc, Ff, DIre, DIim = _noise_matrices()
    c["g_re"], c["g_im"] = f32(Gc.real), f32(Gc.imag)          # [65,129]
    c["d_re"], c["d_im"] = f32(Ff.real), f32(Ff.imag)          # [64,129]
    c["di_re"], c["di_im"] = f32(DIre), f32(DIim)              # [129,256]
    n1 = np.arange(256)
    k1 = np.arange(256)
    n2 = np.arange(512)
    W256 = np.exp(-2j * np.pi * np.outer(n1, k1) / 256.0)
    W512 = np.exp(-2j * np.pi * np.outer(n2, n2) / 512.0)
    TWf = np.exp(-2j * np.pi * np.outer(k1, n2) / 131072.0)
    TWi = np.exp(2j * np.pi * np.outer(n2, k1) / 131072.0)
    isc = 0.25 / 131072.0
    c["w256c"], c["w256s"] = f32(W256.real), f32(W256.imag)
    c["w512c"], c["w512s"] = f32(W512.real), f32(W512.imag)
    c["tw2c"] = f32(np.cos(2 * np.pi * n2 / 512.0)).reshape(512, 1)
    c["tw2s"] = f32(np.sin(2 * np.pi * n2 / 512.0)).reshape(512, 1)
    c["twfc"], c["twfs"] = f32(TWf.real), f32(TWf.imag)        # [256,512]
    c["twic"], c["twis"] = f32(TWi.real), f32(TWi.imag)        # [512,256]
    c["i2c"] = f32(np.cos(2 * np.pi * np.outer(k1, n1) / 256.0) * isc)   # [k1,n1]
    c["i2s"] = f32(-np.sin(2 * np.pi * np.outer(k1, n1) / 256.0) * isc)
    return {k: np.ascontiguousarray(v, np.float32) for k, v in c.items()}

CONSTS = _make_consts()


def _prep_weights(I):
    f32 = lambda v: np.ascontiguousarray(v, np.float32)
    d = {}
    d["pre_w"] = f32(I["pre_w"])                       # [2,128]
    d["pre_b"] = f32(np.reshape(I["pre_b"], (128, 1)))
    d["gru_k"] = f32(I["gru_k"])                       # [128,768]
    d["gru_rk"] = f32(I["gru_rk"])                     # [256,768]
    d["b0col"] = f32(np.reshape(I["gru_b"][0], (768, 1)))
    d["b1col"] = f32(np.reshape(I["gru_b"][1], (768, 1)))
    d["post_w1"] = f32(I["post_w1"])
    d["pb1col"] = f32(np.reshape(I["post_b1"], (256, 1)))
    d["post_w2"] = f32(I["post_w2"])
    d["pb2col"] = f32(np.reshape(I["post_b2"], (128, 1)))
    hw = np.zeros((128, 256), np.float32)
    hw[:, 0:1] = I["amp_w"]
    hw[:, 1:65] = I["harm_w"]
    hw[:, 65:130] = I["noise_w"]
    d["headsW"] = hw
    hb = np.zeros((256, 1), np.float32)
    hb[0, 0] = np.asarray(I["amp_b"]).reshape(-1)[0]
    hb[1:65, 0] = I["harm_b"]
    hb[65:130, 0] = I["noise_b"]
    d["headsB"] = hb
    return d

W_SHAPES = {
    "pre_w": (2, 128), "pre_b": (128, 1), "gru_k": (128, 768),
    "gru_rk": (256, 768), "b0col": (768, 1), "b1col": (768, 1),
    "post_w1": (256, 256), "pb1col": (256, 1), "post_w2": (256, 128),
    "pb2col": (128, 1), "headsW": (128, 256), "headsB": (256, 1),
}


# ---------------------------------------------------------------------------
# program builder
# ---------------------------------------------------------------------------

class Prog:
    def __init__(self, debug_taps=False):
        self.debug = debug_taps
        self.taps = {}
        nc = bacc.Bacc("TRN2", num_devices=N_CORES, target_bir_lowering=False,
                       debug=False)
        self.nc = nc
        self.din = {}
        for name, shape in [("f0pad", [TP + 32]), ("loudpad", [TP + 32]),
                            ("noisepad", [NSP]), ("rirpad", [NSP])]:
            self.din[name] = nc.dram_tensor(name, shape, F32, kind="ExternalInput")
        for k, s in W_SHAPES.items():
            self.din["w_" + k] = nc.dram_tensor("w_" + k, list(s), F32,
                                                kind="ExternalInput")
        for k, v in CONSTS.items():
            self.din["c_" + k] = nc.dram_tensor("c_" + k, list(v.shape), F32,
                                                kind="ExternalInput")
        self.out = nc.dram_tensor("audio_out", [NS], F32, kind="ExternalOutput")
        self.scr = {}
        for name, shape in [("a_dram", [TP + 8, 64]), ("mags_dram", [TP, NB]),
                            ("delta_dram", [TP]),
                            ("phired_dram", [TP]), ("audio_dram", [NSP]),
                            ("ola_dram", [NSP + 1024]),
                            ("sq_re", [512, 256]), ("sq_im", [512, 256]),
                            ("of_dram", [TP + 8, 256])]:
            self.scr[name] = nc.dram_tensor(name, shape, F32)

    def tap(self, name, ap):
        if not self.debug:
            return
        t = self.nc.dram_tensor("tap_" + name, list(ap.shape), ap.dtype,
                                kind="ExternalOutput")
        self.nc.sync.dma_start(t.ap(), ap)
        self.taps[name] = t

    def build(self):
        nc = self.nc
        with tile.TileContext(nc) as tc:
            self._body(tc)
        nc.compile()
        return nc

    # -------------------------------------------------------------------
    def _body(self, tc):
        from contextlib import ExitStack
        with ExitStack() as ctx:
            self._phase_a(tc, ctx)
        with ExitStack() as ctx:
            self._c_consts(tc, ctx)
            self._phase_b(tc, ctx)
            self._phase_c(tc, ctx)

    # -------------------------------------------------------------------
    def _phase_a(self, tc, ctx):
        nc = self.nc
        din = self.din
        wp = ctx.enter_context(tc.tile_pool(name="wp", bufs=1))
        ap_ = ctx.enter_context(tc.tile_pool(name="actv", bufs=1))
        tp = ctx.enter_context(tc.tile_pool(name="tmpA", bufs=2))
        pp = ctx.enter_context(tc.tile_pool(name="psA", bufs=4, space="PSUM"))

        def wload(name, rows, cols, dtype=F32, rnd=False):
            """Load weight/const dram tensor into <=128-row SBUF tiles."""
            tiles = []
            nchunk = (rows + 127) // 128
            for i in range(nchunk):
                r0, r1 = i * 128, min((i + 1) * 128, rows)
                tl = wp.tile([r1 - r0, cols], dtype, name=f"{name}_{i}")
                if rnd:
                    st = tp.tile([r1 - r0, cols], F32, name=f"{name}_st{i}",
                                 tag="wstage")
                    nc.sync.dma_start(st[:], din[name].ap()[r0:r1])
                    nc.vector.tensor_copy(tl[:], st[:])
                else:
                    nc.sync.dma_start(tl[:], din[name].ap()[r0:r1])
                tiles.append(tl)
            return tiles

        prew_s0 = tp.tile([1, 128], F32, name="prew_s0")
        prew_s1 = tp.tile([1, 128], F32, name="prew_s1")
        nc.sync.dma_start(prew_s0[:], din["w_pre_w"].ap()[0:1])
        nc.sync.dma_start(prew_s1[:], din["w_pre_w"].ap()[1:2])
        prew0 = wp.tile([1, 128], F32R, name="prew0")
        prew1 = wp.tile([1, 128], F32R, name="prew1")
        nc.vector.tensor_copy(prew0[:], prew_s0[:])
        nc.vector.tensor_copy(prew1[:], prew_s1[:])
        pre_b = wload("w_pre_b", 128, 1)[0]
        gru_k = wload("w_gru_k", 128, 768, F32R, rnd=True)[0]
        rk = wload("w_gru_rk", 256, 768, F32R, rnd=True)
        b0 = wload("w_b0col", 768, 1)
        b1 = wload("w_b1col", 768, 1)
        pw1 = wload("w_post_w1", 256, 256, F32R, rnd=True)
        pb1 = wload("w_pb1col", 256, 1)
        pw2 = wload("w_post_w2", 256, 128, F32R, rnd=True)
        pb2 = wload("w_pb2col", 128, 1)[0]
        hW = wload("w_headsW", 128, 256, F32R, rnd=True)[0]
        hB = wload("w_headsB", 256, 1)
        eye = wload("c_eye", 128, 128)[0]
        c8000oh = wload("c_c8000oh", 128, 64)[0]

        ln2col = wp.tile([128, 1], F32, name="ln2col")
        nc.vector.memset(ln2col[:], LN2)
        m5col = wp.tile([128, 1], F32, name="m5col")
        nc.vector.memset(m5col[:], -5.0)

        # ---- f0m / loud rows [1, TP] ----
        f0row = tp.tile([1, TP], F32, name="f0row")
        loudrow = tp.tile([1, TP], F32, name="loudrow")
        nc.sync.dma_start(f0row[:], din["f0pad"].ap()[0:TP].unsqueeze(0))
        nc.sync.dma_start(loudrow[:], din["loudpad"].ap()[0:TP].unsqueeze(0))
        lnf0 = tp.tile([1, TP], F32, name="lnf0")
        nc.scalar.activation(lnf0[:], f0row[:], AFT.Ln)
        a_log = 12.0 / float(np.log(2.0))
        c_log = 69.0 - 12.0 * float(np.log2(440.0))
        f0mR = ap_.tile([1, TP], F32R, name="f0mR")
        loudR = ap_.tile([1, TP], F32R, name="loudR")
        nc.scalar.activation(f0mR[:], lnf0[:], AFT.Copy, scale=a_log,
                             bias=c_log)
        nc.scalar.activation(loudR[:], loudrow[:], AFT.Copy)

        # ---- x1T [128, TP] ----
        x1T = ap_.tile([128, TP], F32R, name="x1T")
        for n in range(2):
            ps = pp.tile([128, 512], F32, name="ps_x1", tag="psa")
            nc.tensor.matmul(ps[:], prew0[:], f0mR[:, n * 512:(n + 1) * 512],
                             start=True, stop=False)
            nc.tensor.matmul(ps[:], prew1[:], loudR[:, n * 512:(n + 1) * 512],
                             start=False, stop=True)
            nc.scalar.activation(x1T[:, n * 512:(n + 1) * 512], ps[:],
                                 AFT.Relu, bias=pre_b[:])
        self.tap("x1T", x1T[:])

        # ---- xpT 6 x [128, TP] ----
        xpT = []
        for m in range(6):
            xm = ap_.tile([128, TP], F32, name=f"xpT{m}")
            for n in range(2):
                ps = pp.tile([128, 512], F32, name="ps_xp", tag="psa")
                nc.tensor.matmul(ps[:], gru_k[:, m * 128:(m + 1) * 128],
                                 x1T[:, n * 512:(n + 1) * 512],
                                 start=True, stop=True)
                nc.scalar.activation(xm[:, n * 512:(n + 1) * 512], ps[:],
                                     AFT.Identity, bias=b0[m][:])
            xpT.append(xm)

        # ---- GRU Picard ----
        zcol = wp.tile([128, 1], F32, name="zcol")
        nc.vector.memset(zcol[:], 0.0)
        hT = []
        for k in range(2):
            ht = ap_.tile([128, TP + 1], F32R, name=f"hT{k}")
            nc.vector.tensor_copy(ht[:, 0:1], zcol[:])
            hT.append(ht)

        gp = ctx.enter_context(tc.tile_pool(name="gru", bufs=1))
        for it in range(GRU_ITERS):
            rpT = {}
            gates = {}
            if it > 0:
                for m in range(6):
                    if m < 4:
                        g = gp.tile([128, TP], F32, name=f"gin{m}", tag="gin",
                                    bufs=4)
                        gates[m] = g
                    else:
                        g = gp.tile([128, TP], F32, name=f"rpT{m}",
                                    tag=f"rp{m}")
                        rpT[m] = g
                    for n in range(2):
                        ps = pp.tile([128, 512], F32, name="ps_rp", tag="psa")
                        for k in range(2):
                            nc.tensor.matmul(
                                ps[:], rk[k][:, m * 128:(m + 1) * 128],
                                hT[k][:, n * 512:n * 512 + 512],
                                start=(k == 0), stop=(k == 1))
                        if m < 4:
                            nc.vector.scalar_tensor_tensor(
                                g[:, n * 512:(n + 1) * 512], ps[:], b1[m][:],
                                xpT[m][:, n * 512:(n + 1) * 512],
                                OP.add, OP.add)
                        else:
                            nc.scalar.activation(g[:, n * 512:(n + 1) * 512],
                                                 ps[:], AFT.Identity,
                                                 bias=b1[m][:])

            def gate_in(m, name):
                if it > 0:
                    return gates[m]
                g = gp.tile([128, TP], F32, name=name, tag="gin", bufs=4)
                nc.vector.tensor_scalar(g[:], xpT[m][:], b1[m][:], None,
                                        OP.add)
                return g

            zT, rT, cT = [], [], []
            for k in range(2):
                zin = gate_in(0 + k, f"zin{k}")
                z = gp.tile([128, TP], F32, name=f"zT{k}", tag=f"zT{k}")
                nc.scalar.activation(z[:], zin[:], AFT.Sigmoid)
                zT.append(z)
            for k in range(2):
                rin = gate_in(2 + k, f"rin{k}")
                r = gp.tile([128, TP], F32, name=f"rT{k}", tag=f"rT{k}")
                nc.scalar.activation(r[:], rin[:], AFT.Sigmoid)
                rT.append(r)
            for k in range(2):
                rrh = gp.tile([128, TP], F32, name=f"rrh{k}", tag="gtmp", bufs=2)
                if it == 0:
                    nc.vector.tensor_scalar(rrh[:], rT[k][:], b1[4 + k][:],
                                            None, OP.mult)
                else:
                    nc.vector.tensor_tensor(rrh[:], rT[k][:], rpT[4 + k][:],
                                            OP.mult)
                hin = gp.tile([128, TP], F32, name=f"hin{k}", tag="gin", bufs=4)
                nc.vector.tensor_tensor(hin[:], rrh[:], xpT[4 + k][:], OP.add)
                hh = gp.tile([128, TP], F32, name=f"hh{k}", tag=f"hh{k}")
                nc.scalar.activation(hh[:], hin[:], AFT.Tanh)
                # negcc = (z - 1) * hh ; h = z*h_prev - negcc == z*h + (1-z)*hh
                cc = gp.tile([128, TP], F32, name=f"cT{k}", tag=f"cT{k}")
                nc.vector.scalar_tensor_tensor(cc[:], zT[k][:], 1.0, hh[:],
                                               OP.subtract, OP.mult)
                cT.append(cc)
            for k in range(2):
                nc.vector.tensor_tensor_scan(hT[k][:, 1:TP + 1], zT[k][:],
                                             cT[k][:], 0.0, OP.mult,
                                             OP.subtract)
        self.tap("hT0", hT[0][:])

        # ---- post MLP ----
        x2T = []
        for m in range(2):
            xm = ap_.tile([128, TP], F32R, name=f"x2T{m}")
            for n in range(2):
                ps = pp.tile([128, 512], F32, name="ps_x2", tag="psa")
                for k in range(2):
                    nc.tensor.matmul(ps[:], pw1[k][:, m * 128:(m + 1) * 128],
                                     hT[k][:, 1 + n * 512:1 + n * 512 + 512],
                                     start=(k == 0), stop=(k == 1))
                nc.scalar.activation(xm[:, n * 512:(n + 1) * 512], ps[:],
                                     AFT.Relu, bias=pb1[m][:])
            x2T.append(xm)
        x3T = ap_.tile([128, TP], F32R, name="x3T")
        for n in range(2):
            ps = pp.tile([128, 512], F32, name="ps_x3", tag="psa")
            for k in range(2):
                nc.tensor.matmul(ps[:], pw2[k][:],
                                 x2T[k][:, n * 512:(n + 1) * 512],
                                 start=(k == 0), stop=(k == 1))
            nc.scalar.activation(x3T[:, n * 512:(n + 1) * 512], ps[:],
                                 AFT.Relu, bias=pb2[:])
        headsT = []
        for m in range(2):
            hm = ap_.tile([128, TP], F32, name=f"headsT{m}")
            for n in range(2):
                ps = pp.tile([128, 512], F32, name="ps_hd", tag="psa")
                nc.tensor.matmul(ps[:], hW[:, m * 128:(m + 1) * 128],
                                 x3T[:, n * 512:(n + 1) * 512],
                                 start=True, stop=True)
                nc.scalar.activation(hm[:, n * 512:(n + 1) * 512], ps[:],
                                     AFT.Identity, bias=hB[m][:])
            headsT.append(hm)
        self.tap("headsT0", headsT[0][:])

        # ---- transpose heads to frame-major; batched control math ----
        cp = ctx.enter_context(tc.tile_pool(name="ctlF", bufs=1))
        sgA = cp.tile([128, 512], F32, name="sgA")     # (c,h) harm logits
        amA = cp.tile([128, 8], F32, name="amA")       # amp head per chunk
        mgA = cp.tile([128, 520], F32, name="mgA")     # (c,band) noise head
        for c in range(NCH):
            ps0 = pp.tile([128, 128], F32, name="ps_tr", tag="pst")
            nc.tensor.transpose(ps0[:], headsT[0][:, c * 128:(c + 1) * 128],
                                eye[:])
            nc.scalar.activation(amA[:, c:c + 1], ps0[:, 0:1], AFT.Copy)
            nc.scalar.activation(sgA[:, c * 64:(c + 1) * 64], ps0[:, 1:65],
                                 AFT.Copy)
            nc.scalar.activation(mgA[:, c * 65:c * 65 + 63], ps0[:, 65:128],
                                 AFT.Copy)
            ps1 = pp.tile([128, 128], F32, name="ps_tr2", tag="pst")
            nc.tensor.transpose(ps1[:], headsT[1][:, c * 128:(c + 1) * 128],
                                eye[:])
            nc.scalar.activation(mgA[:, c * 65 + 63:c * 65 + 65], ps1[:, 0:2],
                                 AFT.Copy)
        sgA3 = sgA[:].rearrange("p (c h) -> p c h", h=64)
        hmaxA = cp.tile([128, 8], F32, name="hmaxA")
        nc.vector.tensor_reduce(hmaxA[:], sgA3, AX, OP.max)
        hmax_b = hmaxA[:].unsqueeze(2).broadcast_to([128, 8, 64])
        nc.vector.tensor_tensor(sgA3, sgA3, hmax_b, OP.subtract)
        nc.scalar.activation(sgA[:], sgA[:], AFT.Exp)
        esumA = cp.tile([128, 8], F32, name="esumA")
        nc.vector.tensor_reduce(esumA[:], sgA3, AX, OP.add)
        erecA = cp.tile([128, 8], F32, name="erecA")
        nc.vector.reciprocal(erecA[:], esumA[:])
        erec_b = erecA[:].unsqueeze(2).broadcast_to([128, 8, 64])
        nc.vector.tensor_tensor(sgA3, sgA3, erec_b, OP.mult)
        # exp_sigmoid chains, batched over all chunks
        nc.scalar.activation(sgA[:], sgA[:], AFT.Sigmoid)
        nc.scalar.activation(amA[:], amA[:], AFT.Sigmoid)
        nc.scalar.activation(mgA[:], mgA[:], AFT.Sigmoid)
        nc.scalar.activation(sgA[:], sgA[:], AFT.Ln)
        nc.scalar.activation(amA[:], amA[:], AFT.Ln)
        nc.scalar.activation(mgA[:], mgA[:], AFT.Ln)
        nc.scalar.activation(sgA[:], sgA[:], AFT.Exp, scale=LN10,
                             bias=ln2col[:])
        nc.scalar.activation(amA[:], amA[:], AFT.Exp, scale=LN10,
                             bias=ln2col[:])
        nc.scalar.activation(mgA[:], mgA[:], AFT.Exp, scale=LN10,
                             bias=ln2col[:])
        nc.scalar.activation(amA[:], amA[:], AFT.Sigmoid)
        nc.scalar.activation(mgA[:], mgA[:], AFT.Sigmoid, bias=m5col[:])
        nc.scalar.activation(amA[:], amA[:], AFT.Ln)
        nc.scalar.activation(mgA[:], mgA[:], AFT.Ln)
        nc.scalar.activation(amA[:], amA[:], AFT.Exp, scale=LN10,
                             bias=ln2col[:])
        nc.scalar.activation(mgA[:], mgA[:], AFT.Exp, scale=LN10,
                             bias=ln2col[:])
        # nyquist mask, normalize, a = amp*hd; a/mags out (single DMAs)
        f0A = cp.tile([128, 8], F32, name="f0A")
        nc.sync.dma_start(f0A[:], din["f0pad"].ap()[0:1024]
                          .rearrange("(c p) -> p c", c=8))
        mskA = cp.tile([128, 512], F32, name="mskA")
        mskA3 = mskA[:].rearrange("p (c h) -> p c h", h=64)
        c8_b = c8000oh[:].unsqueeze(1).broadcast_to([128, 8, 64])
        f0_b = f0A[:].unsqueeze(2).broadcast_to([128, 8, 64])
        nc.vector.tensor_tensor(mskA3, c8_b, f0_b, OP.is_gt)
        nc.vector.tensor_tensor(sgA[:], sgA[:], mskA[:], OP.mult)
        denA = cp.tile([128, 8], F32, name="denA")
        nc.vector.tensor_reduce(denA[:], sgA3, AX, OP.add)
        dreA = cp.tile([128, 8], F32, name="dreA")
        nc.vector.reciprocal(dreA[:], denA[:])
        dre_b = dreA[:].unsqueeze(2).broadcast_to([128, 8, 64])
        nc.vector.tensor_tensor(sgA3, sgA3, dre_b, OP.mult)
        aFA = cp.tile([128, 512], F32, name="aFA")
        aFA3 = aFA[:].rearrange("p (c h) -> p c h", h=64)
        am_b = amA[:].unsqueeze(2).broadcast_to([128, 8, 64])
        nc.vector.tensor_tensor(aFA3, sgA3, am_b, OP.mult)
        nc.sync.dma_start(self.scr["a_dram"].ap()[0:1024]
                          .rearrange("(c p) h -> p c h", c=8),
                          aFA[:].rearrange("p (c h) -> p c h", c=8))
        nc.sync.dma_start(self.scr["mags_dram"].ap()[0:TP]
                          .rearrange("(c p) n -> p c n", c=8),
                          mgA[:].rearrange("p (c n) -> p c n", c=8))
        self.tap("aFA", aFA[:])
        # frame-999 interp fix: a_dram[1000] <- a_dram[999]
        afix = cp.tile([1, 64], F32, name="afix", tag="afix")
        nc.sync.dma_start(afix[:], self.scr["a_dram"].ap()[999:1000])
        nc.sync.dma_start(self.scr["a_dram"].ap()[1000:1001], afix[:])


    # -------------------------------------------------------------------
    def _phase_b(self, tc, ctx):
        """Harmonic synthesis -> audio_dram[0:NSP] (harmonic part)."""
        nc = self.nc
        din = self.din
        wp = ctx.enter_context(tc.tile_pool(name="wpB", bufs=1))
        fp_ = ctx.enter_context(tc.tile_pool(name="frameB", bufs=1))
        from contextlib import ExitStack
        sp = ctx.enter_context(tc.tile_pool(name="smallB", bufs=2))
        hctx = ExitStack()

        def cload(name, rows, cols):
            tl = wp.tile([rows, cols], F32, name=name)
            nc.sync.dma_start(tl[:], din["c_" + name].ap()[0:rows])
            return tl

        w0ts = cload("w0ts", 128, 64)
        w1ts = cload("w1ts", 128, 64)
        h2row = cload("h2row", 128, 64)
        pioh = cload("pioh", 128, 64)
        ones = cload("ones", 128, TP)

        # ---- per-chunk dphi / p; frame increments to delta_dram ----
        dphis, ps_ = [], []
        for c in range(NCH):
            f0c = sp.tile([128, 1], F32, name="f0c", tag="f0c")
            f0n = sp.tile([128, 1], F32, name="f0n", tag="f0n")
            nc.sync.dma_start(f0c[:], din["f0pad"].ap()
                              [c * 128:c * 128 + 128].unsqueeze(1))
            nc.sync.dma_start(f0n[:], din["f0pad"].ap()
                              [c * 128 + 1:c * 128 + 129].unsqueeze(1))
            dp = fp_.tile([128, 64], F32, name=f"dphi{c}")
            nc.vector.tensor_scalar(dp[:], w1ts[:], f0n[:], None, OP.mult)
            nc.vector.scalar_tensor_tensor(dp[:], w0ts[:], f0c[:], dp[:],
                                           OP.mult, OP.add)
            pc = fp_.tile([128, 64], F32, name=f"p{c}")
            nc.vector.tensor_tensor_scan(pc[:], ones[:, 0:64], dp[:], 0.0,
                                         OP.mult, OP.add)
            nc.sync.dma_start(self.scr["delta_dram"].ap()
                              [c * 128:(c + 1) * 128].unsqueeze(1),
                              pc[:, 63:64])
            dphis.append(dp)
            ps_.append(pc)

        # ---- frame-level compensated phase scan ----
        drow = fp_.tile([1, TP], F32, name="drow")
        nc.sync.dma_start(drow[:], self.scr["delta_dram"].ap()[0:TP].unsqueeze(0))
        zc1 = fp_.tile([1, 1], F32, name="zc1")
        nc.vector.memset(zc1[:], 0.0)
        hi = fp_.tile([1, TP + 1], F32, name="hi")
        nc.vector.tensor_copy(hi[:, 0:1], zc1[:])
        nc.vector.tensor_tensor_scan(hi[:, 1:TP + 1], ones[0:1, :], drow[:],
                                     0.0, OP.mult, OP.add)
        dd = fp_.tile([1, TP], F32, name="dd")
        nc.vector.tensor_tensor(dd[:], hi[:, 1:TP + 1], hi[:, 0:TP],
                                OP.subtract)
        ee = fp_.tile([1, TP], F32, name="ee")
        nc.vector.tensor_tensor(ee[:], drow[:], dd[:], OP.subtract)
        lo = fp_.tile([1, TP + 1], F32, name="lo")
        nc.vector.tensor_copy(lo[:, 0:1], zc1[:])
        nc.vector.tensor_tensor_scan(lo[:, 1:TP + 1], ones[0:1, :], ee[:],
                                     0.0, OP.mult, OP.add)
        kff = fp_.tile([1, TP], F32, name="kff")
        nc.vector.tensor_scalar(kff[:], hi[:, 0:TP], INV2PI, MAGIC, OP.mult,
                                OP.add)
        nc.vector.tensor_scalar(kff[:], kff[:], MAGIC, None, OP.subtract)
        cwf = fp_.tile([1, TP], F32, name="cwf")
        nc.vector.cody_waite_cascade(cwf[:], hi[:, 0:TP], kff[:], *CW_FRAME)
        phr = fp_.tile([1, TP], F32, name="phr")
        nc.vector.tensor_tensor(phr[:], cwf[:], lo[:, 0:TP], OP.add)
        self.tap("phired", phr[:])
        nc.sync.dma_start(self.scr["phired_dram"].ap()[0:TP].unsqueeze(0),
                          phr[:])

        # ---- noise path: per-frame FIR in frequency domain ----
        eye = cload("eye", 128, 128)
        gre = cload("g_re", 65, 129)
        gim = cload("g_im", 65, 129)
        dre = cload("d_re", 64, 129)
        dim_ = cload("d_im", 64, 129)
        di_re0 = cload("di_re", 128, 256)
        di_im0 = cload("di_im", 128, 256)
        di_re1 = wp.tile([1, 256], F32, name="di_re1")
        nc.sync.dma_start(di_re1[:], din["c_di_re"].ap()[128:129])
        di_im1 = wp.tile([1, 256], F32, name="di_im1")
        nc.sync.dma_start(di_im1[:], din["c_di_im"].ap()[128:129])
        rnd = {}
        for nm, src in [("gre", gre), ("gim", gim), ("dre", dre),
                        ("dim", dim_), ("dire0", di_re0), ("diim0", di_im0),
                        ("dire1", di_re1), ("diim1", di_im1)]:
            dst = wp.tile(list(src.shape), F16, name=nm + "_r")
            nc.vector.tensor_copy(dst[:], src[:])
            rnd[nm] = dst

        np_ = ctx.enter_context(tc.tile_pool(name="noiseB", bufs=2))
        pp = ctx.enter_context(tc.tile_pool(name="psB", bufs=2, space="PSUM"))
        zt = wp.tile([128, 256], F32, name="zt")
        nc.vector.memset(zt[:], 0.0)
        nc.sync.dma_start(self.scr["of_dram"].ap()[0:4], zt[0:4, :])
        nc.sync.dma_start(self.scr["of_dram"].ap()[4 + TP:8 + TP], zt[0:4, :])

        for sc_ in range(4):
            magsT = np_.tile([128, 256], F16, name="magsT", tag="magsT")
            frT = np_.tile([128, 256], F16, name="frT", tag="frT")
            for half in range(2):
                c = sc_ * 2 + half
                mstage = np_.tile([128, 128], F32, name="mstage", tag="mstage")
                nc.vector.memset(mstage[:], 0.0)
                nc.sync.dma_start(mstage[:, 0:NB], self.scr["mags_dram"].ap()
                                  [c * 128:(c + 1) * 128])
                pst = pp.tile([128, 128], F32, name="pst", tag="pst")
                nc.tensor.transpose(pst[:], mstage[:], eye[:])
                nc.scalar.activation(magsT[:, half * 128:(half + 1) * 128],
                                     pst[:], AFT.Copy)
                fstage = np_.tile([128, 128], F32, name="fstage", tag="fstage")
                nc.vector.memset(fstage[:], 0.0)
                nc.sync.dma_start(
                    fstage[:, 0:64], din["noisepad"].ap()
                    [c * 8192:(c + 1) * 8192].rearrange("(p j) -> p j", j=64))
                pst2 = pp.tile([128, 128], F32, name="pst2", tag="pst")
                nc.tensor.transpose(pst2[:], fstage[:], eye[:])
                nc.scalar.activation(frT[:, half * 128:(half + 1) * 128],
                                     pst2[:], AFT.Copy)
            # spectra [129, 256] as (128-row tile, 1-row tile) pairs
            def spectrum(nm, lhsT, kdim, rhs):
                big = np_.tile([128, 256], F16, name=nm + "b", tag=nm + "b")
                ps0 = pp.tile([128, 256], F32, name="ps_sp", tag="pss")
                nc.tensor.matmul(ps0[:], lhsT[0:kdim, 0:128], rhs[0:kdim, :],
                                 start=True, stop=True)
                nc.scalar.activation(big[:], ps0[:], AFT.Copy)
                one = np_.tile([1, 256], F16, name=nm + "o", tag=nm + "o")
                ps1 = pp.tile([1, 256], F32, name="ps_sp1", tag="pss1")
                nc.tensor.matmul(ps1[:], lhsT[0:kdim, 128:129], rhs[0:kdim, :],
                                 start=True, stop=True)
                nc.scalar.activation(one[:], ps1[:], AFT.Copy)
                return big, one
            irre = spectrum("irre", rnd["gre"], 65, magsT)
            irim = spectrum("irim", rnd["gim"], 65, magsT)
            frre = spectrum("frre", rnd["dre"], 64, frT)
            frim = spectrum("frim", rnd["dim"], 64, frT)
            # P = FR * IR (complex), f32r
            pres, pims = [], []
            for idx in range(2):
                fre, fim = frre[idx], frim[idx]
                ire, iim = irre[idx], irim[idx]
                rows = fre.shape[0]
                t1 = np_.tile([rows, 256], F16, name="t1", tag=f"t1_{idx}")
                t2_ = np_.tile([rows, 256], F16, name="t2n", tag=f"t2n_{idx}")
                pre = np_.tile([rows, 256], F16, name="pre", tag=f"pre_{idx}")
                pim = np_.tile([rows, 256], F16, name="pim", tag=f"pim_{idx}")
                nc.vector.tensor_tensor(t1[:], fre[:], ire[:], OP.mult)
                nc.vector.tensor_tensor(t2_[:], fim[:], iim[:], OP.mult)
                nc.vector.tensor_tensor(pre[:], t1[:], t2_[:], OP.subtract)
                nc.vector.tensor_tensor(t1[:], fre[:], iim[:], OP.mult)
                nc.vector.tensor_tensor(t2_[:], fim[:], ire[:], OP.mult)
                nc.vector.tensor_tensor(pim[:], t1[:], t2_[:], OP.add)
                pres.append(pre)
                pims.append(pim)
            # outfullT [256u (2 chunks), 256t] then transpose to of_dram rows
            for mu in range(2):
                pso = pp.tile([128, 256], F32, name="ps_of", tag="pss")
                nc.tensor.matmul(pso[:], rnd["dire0"][:, mu * 128:(mu + 1) * 128],
                                 pres[0][:], start=True, stop=False)
                nc.tensor.matmul(pso[:], rnd["dire1"][:, mu * 128:(mu + 1) * 128],
                                 pres[1][:], start=False, stop=False)
                nc.tensor.matmul(pso[:], rnd["diim0"][:, mu * 128:(mu + 1) * 128],
                                 pims[0][:], start=False, stop=False)
                nc.tensor.matmul(pso[:], rnd["diim1"][:, mu * 128:(mu + 1) * 128],
                                 pims[1][:], start=False, stop=True)
                ofT = np_.tile([128, 256], F32, name="ofT", tag="ofT")
                nc.scalar.activation(ofT[:], pso[:], AFT.Copy)
                for th_ in range(2):
                    psb = pp.tile([128, 128], F32, name="ps_tb", tag="pst")
                    nc.tensor.transpose(psb[:], ofT[:, th_ * 128:(th_ + 1) * 128],
                                        eye[:])
                    ofb = np_.tile([128, 128], F32, name="ofb", tag="ofb")
                    nc.scalar.activation(ofb[:], psb[:], AFT.Copy)
                    row0 = 4 + sc_ * 256 + th_ * 128
                    nc.sync.dma_start(
                        self.scr["of_dram"].ap()
                        [row0:row0 + 128, mu * 128:(mu + 1) * 128], ofb[:])

        # ---- heavy per-chunk loop: fused phase-frac, mask-select, amp ----
        hp = hctx.enter_context(tc.tile_pool(name="heavyB", bufs=1))
        ahsA = fp_.tile([128, 512], F32, name="ahsA")
        for c in range(NCH):
            phic = sp.tile([128, 1], F32, name="phic", tag="phic")
            nc.sync.dma_start(phic[:], self.scr["phired_dram"].ap()
                              [c * 128:(c + 1) * 128].unsqueeze(1))
            # f = u - round(u), u = (P_j + phi_c) * h/(2pi)
            f_ = hp.tile([128, 64 * 64], FP16, name="f_", tag="f_", bufs=2)
            f3 = f_[:].rearrange("p (j h) -> p j h", h=64)
            p_b = ps_[c][:].unsqueeze(2).broadcast_to([128, 64, 64])
            h2_b = h2row[:].unsqueeze(1).broadcast_to([128, 64, 64])
            nc.vector._custom_dve(OP_PHASE, out=f3, in0=p_b, in1=h2_b,
                                  s0=phic[:], s1=MAGIC)
            s16 = hp.tile([128, 64 * 64], F16, name="s16", tag="s16", bufs=2)
            nc.scalar.activation(s16[:], f_[:], AFT.Sin, scale=TWO_PI)
            msk = hp.tile([128, 64 * 64], F16, name="msk", tag="msk", bufs=2)
            d_b = dphis[c][:].unsqueeze(2).broadcast_to([128, 64, 64])
            pio_b = pioh[:].unsqueeze(1).broadcast_to([128, 64, 64])
            nc.vector.tensor_tensor(msk[:].rearrange("p (j h) -> p j h", h=64),
                                    d_b, pio_b, OP.is_lt)
            aC = sp.tile([128, 64], F32, name="aC", tag="aC")
            aN = sp.tile([128, 64], F32, name="aN", tag="aN")
            nc.sync.dma_start(aC[:], self.scr["a_dram"].ap()
                              [c * 128:c * 128 + 128])
            nc.sync.dma_start(aN[:], self.scr["a_dram"].ap()
                              [c * 128 + 1:c * 128 + 129])
            aC16 = sp.tile([128, 64], F16, name="aC16", tag="aC16")
            aN16 = sp.tile([128, 64], F16, name="aN16", tag="aN16")
            nc.vector.tensor_copy(aC16[:], aC[:])
            nc.vector.tensor_copy(aN16[:], aN[:])
            ae = hp.tile([128, 64 * 64], F16, name="ae", tag="ae", bufs=2)
            ae3 = ae[:].rearrange("p (j h) -> p j h", h=64)
            aC_b = aC16[:].unsqueeze(1).broadcast_to([128, 64, 64])
            aN_b = aN16[:].unsqueeze(1).broadcast_to([128, 64, 64])
            nc.vector._custom_dve(OP_AMP2, out=ae3, in0=aC_b, in1=aN_b,
                                  s0=0.0, s1=1.0 / 64.0)
            nc.vector.tensor_tensor(s16[:], s16[:], ae[:], OP.mult)
            nc.vector.tensor_tensor(s16[:], s16[:], msk[:], OP.mult)
            # one tree level over h, then reduce
            s3 = s16[:].rearrange("p (j h) -> p j h", h=64)
            lv = hp.tile([128, 64 * 32], F16, name="lv", tag="ae", bufs=2)
            lv3 = lv[:].rearrange("p (j h) -> p j h", h=32)
            nc.vector.tensor_tensor(lv3, s3[:, :, 0:32], s3[:, :, 32:64],
                                    OP.add)
            nc.vector.tensor_reduce(ahsA[:, c * 64:(c + 1) * 64], lv3,
                                    AX, OP.add)
            if c == 0:
                self.tap("audio_h0", ahsA[:, 0:64])

        hctx.close()

        # ---- overlap-add + crop + combine with harmonic (batched) ----
        OF = self.scr["of_dram"].ap()
        accA = np_.tile([128, 512], F32, name="accA", tag="accA")
        taA = np_.tile([128, 512], F32, name="taA", tag="taA")
        nc.sync.dma_start(accA[:].rearrange("p (c f) -> p c f", c=8),
                          OF[4:1028, 62:126].rearrange("(c p) f -> p c f", c=8))
        nc.sync.dma_start(taA[:].rearrange("p (c f) -> p c f", c=8),
                          OF[3:1027, 126:190].rearrange("(c p) f -> p c f", c=8))
        nc.vector.tensor_tensor(accA[:], accA[:], taA[:], OP.add)
        taB = np_.tile([128, 512], F32, name="taB", tag="taB")
        nc.sync.dma_start(taB[:].rearrange("p (c f) -> p c f", c=8),
                          OF[2:1026, 190:254].rearrange("(c p) f -> p c f", c=8))
        nc.vector.tensor_tensor(accA[:], accA[:], taB[:], OP.add)
        t3s, t4s = [], []
        for c in range(NCH):
            r0 = 4 + c * 128
            ta3 = np_.tile([128, 2], F32, name=f"ta3_{c}", tag=f"ta3_{c}")
            nc.sync.dma_start(ta3[:], OF[r0 - 3:r0 + 125, 254:256])
            t3s.append(ta3)
            ta4 = np_.tile([128, 62], F32, name=f"ta4_{c}", tag=f"ta4_{c}")
            nc.sync.dma_start(ta4[:], OF[r0 + 1:r0 + 129, 0:62])
            t4s.append(ta4)
        for c in range(NCH):
            nc.vector.tensor_tensor(accA[:, c * 64:c * 64 + 2],
                                    accA[:, c * 64:c * 64 + 2], t3s[c][:],
                                    OP.add)
            nc.vector.tensor_tensor(accA[:, c * 64 + 2:(c + 1) * 64],
                                    accA[:, c * 64 + 2:(c + 1) * 64],
                                    t4s[c][:], OP.add)
        nc.vector.tensor_tensor(accA[:], accA[:], ahsA[:], OP.add)
        nc.sync.dma_start(self.scr["audio_dram"].ap()[0:NSP]
                          .rearrange("(c p j) -> p c j", c=8, j=64),
                          accA[:].rearrange("p (c j) -> p c j", c=8))
        self.tap("audio0", accA[:, 0:64])
        # zero padded tail of audio (samples 64000..65535)
        nc.sync.dma_start(self.scr["audio_dram"].ap()[NS:NSP]
                          .rearrange("(p j) -> p j", j=12), zt[:, 0:12])

    # -------------------------------------------------------------------
    def _c_consts(self, tc, ctx):
        """Load phase-C FFT constants early so their DMAs overlap phase B."""
        nc = self.nc
        din = self.din
        wp = ctx.enter_context(tc.tile_pool(name="wpC", bufs=1))
        tp = ctx.enter_context(tc.tile_pool(name="stC", bufs=2))

        def cload(name, rows, cols, rowoff=0, dt=F32):
            src = din["c_" + name].ap()[rowoff:rowoff + rows]
            if dt != F32:
                st = tp.tile([rows, cols], F32, name=f"{name}{rowoff}_st",
                             tag="cstage")
                nc.sync.dma_start(st[:], src)
                tl = wp.tile([rows, cols], dt, name=f"{name}{rowoff}_r")
                nc.vector.tensor_copy(tl[:], st[:])
            else:
                tl = wp.tile([rows, cols], F32, name=f"{name}{rowoff}")
                nc.sync.dma_start(tl[:], src)
            return tl

        cc = {}
        cc["eye"] = cload("eye", 128, 128)
        eyer = wp.tile([128, 128], F32R, name="eyerC")
        nc.vector.tensor_copy(eyer[:], cc["eye"][:])
        cc["eyer"] = eyer
        cc["w256c"] = [cload("w256c", 128, 256, 128 * i, dt=F32R) for i in range(2)]
        cc["w256s"] = [cload("w256s", 128, 256, 128 * i, dt=F32R) for i in range(2)]
        cc["w512c"] = [cload("w512c", 128, 512, 128 * i, dt=F32R) for i in range(4)]
        cc["w512s"] = [cload("w512s", 128, 512, 128 * i, dt=F32R) for i in range(4)]
        cc["twfc"] = [cload("twfc", 128, 512, 128 * i) for i in range(2)]
        cc["twfs"] = [cload("twfs", 128, 512, 128 * i) for i in range(2)]
        cc["twic"] = [cload("twic", 128, 256, 128 * i) for i in range(4)]
        cc["twis"] = [cload("twis", 128, 256, 128 * i) for i in range(4)]
        cc["i2c"] = [cload("i2c", 128, 256, 128 * i, dt=F32R) for i in range(2)]
        cc["i2s"] = [cload("i2s", 128, 256, 128 * i, dt=F32R) for i in range(2)]
        cc["tw2c"] = [cload("tw2c", 128, 1, 128 * i) for i in range(4)]
        cc["tw2s"] = [cload("tw2s", 128, 1, 128 * i) for i in range(4)]
        self._cc = cc

    # -------------------------------------------------------------------
    def _phase_c(self, tc, ctx):
        """Reverb: 131072-pt FFT convolution; adds dry; writes audio_out."""
        nc = self.nc
        din = self.din
        wp = ctx.enter_context(tc.tile_pool(name="wpC", bufs=1))
        dp_ = ctx.enter_context(tc.tile_pool(name="datC", bufs=1))
        tp = ctx.enter_context(tc.tile_pool(name="tmpC", bufs=2))
        pp = ctx.enter_context(tc.tile_pool(name="psC", bufs=2, space="PSUM"))

        def cload(name, rows, cols, rowoff=0, rnd=False):
            src = din["c_" + name].ap()[rowoff:rowoff + rows]
            if rnd:
                st = tp.tile([rows, cols], F32, name=f"{name}{rowoff}_st",
                             tag="cstage")
                nc.sync.dma_start(st[:], src)
                tl = wp.tile([rows, cols], F32R, name=f"{name}{rowoff}_r")
                nc.vector.tensor_copy(tl[:], st[:])
            else:
                tl = wp.tile([rows, cols], F32, name=f"{name}{rowoff}")
                nc.sync.dma_start(tl[:], src)
            return tl

        eye = cload("eye", 128, 128)
        eyer = wp.tile([128, 128], F32R, name="eyer")
        nc.vector.tensor_copy(eyer[:], eye[:])
        w256c = [cload("w256c", 128, 256, 128 * i, rnd=True) for i in range(2)]
        w256s = [cload("w256s", 128, 256, 128 * i, rnd=True) for i in range(2)]
        w512c = [cload("w512c", 128, 512, 128 * i, rnd=True) for i in range(4)]
        w512s = [cload("w512s", 128, 512, 128 * i, rnd=True) for i in range(4)]
        twfc = [cload("twfc", 128, 512, 128 * i) for i in range(2)]
        twfs = [cload("twfs", 128, 512, 128 * i) for i in range(2)]
        twic = [cload("twic", 128, 256, 128 * i) for i in range(4)]
        twis = [cload("twis", 128, 256, 128 * i) for i in range(4)]
        i2c = [cload("i2c", 128, 256, 128 * i, rnd=True) for i in range(2)]
        i2s = [cload("i2s", 128, 256, 128 * i, rnd=True) for i in range(2)]
        tw2c = [cload("tw2c", 128, 1, 128 * i) for i in range(4)]
        tw2s = [cload("tw2s", 128, 1, 128 * i) for i in range(4)]

        AD = self.scr["audio_dram"].ap().rearrange("(a b) -> a b", b=512)
        RI = din["rirpad"].ap().rearrange("(a b) -> a b", b=512)
        # z rows 128..255 are zero padding: single n1-chunk suffices
        st = tp.tile([128, 512], F32, name="zst", tag="zst")
        nc.sync.dma_start(st[:], AD[0:128])
        zre = dp_.tile([128, 512], F32R, name="zre0")
        nc.vector.tensor_copy(zre[:], st[:])
        st2 = tp.tile([128, 512], F32, name="zst2", tag="zst")
        nc.sync.dma_start(st2[:], RI[0:128])
        zim = dp_.tile([128, 512], F32R, name="zim0")
        nc.vector.tensor_copy(zim[:], st2[:])

        # ---- stage A + forward twiddle -> Y2 [k1(2), 512] f32r ----
        Y2re, Y2im = [], []
        for mk in range(2):
            pa = pp.tile([128, 512], F32, name="pa", tag="pa")
            pb = pp.tile([128, 512], F32, name="pb", tag="pb")
            pc = pp.tile([128, 512], F32, name="pc", tag="pc")
            nc.tensor.matmul(pa[:], w256c[0][:, mk * 128:(mk + 1) * 128],
                             zre[:], start=True, stop=True)
            nc.tensor.matmul(pb[:], w256s[0][:, mk * 128:(mk + 1) * 128],
                             zim[:], start=True, stop=True)
            nc.tensor.matmul(pc[:], w256s[0][:, mk * 128:(mk + 1) * 128],
                             zre[:], start=True, stop=False)
            nc.tensor.matmul(pc[:], w256c[0][:, mk * 128:(mk + 1) * 128],
                             zim[:], start=False, stop=True)
            pbs = tp.tile([128, 512], F32, name="pbs", tag="pbs")
            nc.scalar.activation(pbs[:], pb[:], AFT.Copy)
            yre = tp.tile([128, 512], F32, name="yre", tag="yre")
            nc.vector.tensor_tensor(yre[:], pa[:], pbs[:], OP.subtract)
            yim = tp.tile([128, 512], F32, name="yim", tag="yim")
            nc.scalar.activation(yim[:], pc[:], AFT.Copy)
            t1 = tp.tile([128, 512], F32, name="t1c", tag="t1c")
            t2 = tp.tile([128, 512], F32, name="t2c", tag="t2c")
            y2r = dp_.tile([128, 512], F32R, name=f"y2re{mk}")
            y2i = dp_.tile([128, 512], F32R, name=f"y2im{mk}")
            nc.vector.tensor_tensor(t1[:], yre[:], twfc[mk][:], OP.mult)
            nc.vector.tensor_tensor(t2[:], yim[:], twfs[mk][:], OP.mult)
            nc.vector.tensor_tensor(y2r[:], t1[:], t2[:], OP.subtract)
            nc.vector.tensor_tensor(t1[:], yre[:], twfs[mk][:], OP.mult)
            nc.vector.tensor_tensor(t2[:], yim[:], twfc[mk][:], OP.mult)
            nc.vector.tensor_tensor(y2i[:], t1[:], t2[:], OP.add)
            Y2re.append(y2r)
            Y2im.append(y2i)

        # ---- transpose Y2 -> Y2T [n2(4), 256] f32r ----
        Y2Tre = [dp_.tile([128, 256], F32R, name=f"y2tre{i}") for i in range(4)]
        Y2Tim = [dp_.tile([128, 256], F32R, name=f"y2tim{i}") for i in range(4)]
        for src, dstl in [(Y2re, Y2Tre), (Y2im, Y2Tim)]:
            for kc in range(2):
                for nc_ in range(4):
                    pst = pp.tile([128, 128], F32R, name="pstC", tag="pstC")
                    nc.tensor.transpose(pst[:], src[kc][:, nc_ * 128:(nc_ + 1) * 128],
                                        eyer[:])
                    nc.scalar.activation(dstl[nc_][:, kc * 128:(kc + 1) * 128],
                                         pst[:], AFT.Copy)

        # ---- stage B: X[k2(4), 256] ; spectral square S = X^2 ----
        Sre = [dp_.tile([128, 256], F32R, name=f"sre{i}") for i in range(4)]
        Sim = [dp_.tile([128, 256], F32R, name=f"sim{i}") for i in range(4)]
        for mk2 in range(4):
            pa = pp.tile([128, 256], F32, name="pa2", tag="pa")
            pb = pp.tile([128, 256], F32, name="pb2", tag="pb")
            pc = pp.tile([128, 256], F32, name="pc2", tag="pc")
            for k in range(4):
